# revision 4
# baseline (speedup 1.0000x reference)
"""Trainium2 Bass kernel for a 3x3 stride-1 pad-1 Conv2d (NCHW) (optimized)

Problem (hardcoded): x (16, 128, 128, 128) f32, K (3, 3, 128, 256) f32.
The reference reinterprets K's flat buffer as (Cin, kh, kw, Cout) and only
writes output rows/cols 0..124 (the rest of the 128x128 output stays zero).

All-fp16 datapath (PE streams 1 col/cycle for fp16 and fp32r alike, but
fp16 halves LDWEIGHTS time via FWL, halves x DMA, and halves the out DMA).
Host rounds x/K to fp16 and upconverts the fp16 output to f32; the extra
rounding (~4e-4 rel) is far inside the 2e-2 gate and test.py's atol.
- x is DMA'd in uniform 13-row bands so matmuls never starve.
- ~14 dummy matmuls on a memset tile run during the initial DMA window to
  warm the PE's HAM throttle (cold PE streams at half rate, ~N/1.2 ns);
  without them the first ~100 real matmuls pay ~8us of cold penalty.
- Weights ride the scalar-engine (ACT) queue in 3 tap-group chunks, in
  parallel with band 0: the first matmul waits only on taps 0-2.
- Matmuls compute 125 cols (N=500); last row block is single-row.
- Output DMAs ride the ACT queue so image 1's banded x load never queues
  behind image 0's output DMAs.
"""

import numpy as np

import concourse.bacc as bacc
import concourse.mybir as mybir
import concourse.tile as tile
from concourse.bass_utils import run_bass_kernel_spmd

N_CORES = 8
B, CIN, H, W = 16, 128, 128, 128
COUT = 256
BPC = B // N_CORES  # images per core
HP, WP = H + 2, W + 2  # zero-padded plane
VALID = 125  # valid output rows/cols; rest is zero
CCOLS = 125  # computed cols per row
F32 = mybir.dt.float32
FP16 = mybir.dt.float16

BANDS = [(13 * i, 13) for i in range(10)]  # uniform 13-row bands
N_WARM = 14  # dummy matmuls to warm the PE before real work arrives

_NC_CACHE = {}


def _build_nc(reps=1):
    nc = bacc.Bacc()
    x_in = nc.dram_tensor("x", [BPC, CIN, HP, WP], FP16, kind="ExternalInput")
    w_in = nc.dram_tensor("w", [CIN, 9 * COUT], FP16, kind="ExternalInput")
    out_t = nc.dram_tensor("out", [BPC, COUT, H, W], FP16, kind="ExternalOutput")

    # Row blocks: 31 blocks of 4 rows + 1 single-row block (rows 0..124).
    row_blocks = [(rb * 4, 4) for rb in range(31)] + [(124, 1)]

    with tile.TileContext(nc) as tc:
        with (
            tc.tile_pool(name="warmpool", bufs=1) as warmpool,
            tc.tile_pool(name="wpool", bufs=1) as wpool,
            tc.tile_pool(name="xpool", bufs=2) as xpool,
            tc.tile_pool(name="opool", bufs=8) as opool,
            tc.tile_pool(name="pspool", bufs=8, space="PSUM") as pspool,
        ):
            # PE pre-warm: dummy matmuls on a zeroed tile, issued before
            # any DMA-dependent work so they run while inputs stream in.
            # All write the same PSUM tile (PE-serial WAW); real tiles then
            # cycle the pool and only reuse this bank ~15us later.
            warm = warmpool.tile([CIN, 628], FP16)
            nc.gpsimd.memset(warm[:], 0)
            wps = pspool.tile([128, 4, CCOLS], F32, tag="ps")
            for _ in range(N_WARM):
                nc.tensor.matmul(
                    wps[:], warm[:, 0:128], warm[:, 128:628], start=True, stop=True
                )

            w_sb = wpool.tile([CIN, 9 * COUT], FP16)
            # Weights on the ACT queue in tap-group chunks: the first
            # matmuls only need taps 0-2; later chunks land in time.
            for c0, c1 in ((0, 768), (768, 1536), (1536, 2304)):
                nc.scalar.dma_start(out=w_sb[:, c0:c1], in_=w_in[:, c0:c1])

            for b in [b for _ in range(reps) for b in range(BPC)]:
                x_pad = xpool.tile([CIN, HP, WP], FP16)
                # Banded load: matmuls depend only on the bands they read.
                for r0, nr in BANDS:
                    nc.sync.dma_start(
                        out=x_pad[:, r0 : r0 + nr, :],
                        in_=x_in[b, :, r0 : r0 + nr, :],
                    )

                for r, vr in row_blocks:
                    for c2 in range(2):
                        ps = pspool.tile([128, vr, CCOLS], F32, tag="ps")
                        for i, t in enumerate(range(9)):
                            kh, kw = divmod(t, 3)
                            c0 = t * COUT + c2 * 128
                            lhsT = w_sb[:, c0 : c0 + 128]
                            rhs = x_pad[:, r + kh : r + kh + vr, kw : kw + CCOLS]
                            nc.tensor.matmul(
                                ps[:],
                                lhsT,
                                rhs,
                                start=(i == 0),
                                stop=(i == 8),
                            )
                        ob = opool.tile([128, vr, CCOLS], FP16)
                        nc.vector.tensor_copy(out=ob[:], in_=ps[:])
                        nc.scalar.dma_start(
                            out=out_t[b, c2 * 128 : (c2 + 1) * 128, r : r + vr, 0:CCOLS],
                            in_=ob[:],
                        )
    # Bacc defers register allocation and wait-splitting to compile(),
    # which finalize() runs; the SPMD exec path expects it done already.
    nc.finalize()
    return nc


def _get_nc(reps=1):
    if reps not in _NC_CACHE:
        _NC_CACHE[reps] = _build_nc(reps)
    return _NC_CACHE[reps]


def _run(x, K, trace=False, reps=1):
    x_pad = np.zeros((B, CIN, HP, WP), dtype=np.float16)
    x_pad[:, :, 1 : H + 1, 1 : W + 1] = np.asarray(x, dtype=np.float32).astype(
        np.float16
    )
    # Reference reinterprets K's flat buffer as (Cin, kh, kw, Cout); flat
    # (128, 2304) rows are Cin, cols are (kh*3+kw)*256 + cout.
    w_host = (
        np.ascontiguousarray(np.asarray(K, dtype=np.float32))
        .reshape(CIN, 9 * COUT)
        .astype(np.float16)
    )
    in_maps = [
        {"x": x_pad[i * BPC : (i + 1) * BPC], "w": w_host} for i in range(N_CORES)
    ]
    res = run_bass_kernel_spmd(
        _get_nc(reps), in_maps, list(range(N_CORES)), trace=trace
    )
    out = np.concatenate(
        [res.results[i]["out"] for i in range(N_CORES)], axis=0
    ).astype(np.float32)
    # Device only writes the valid 125x125 region; zero the border strips.
    out[:, :, VALID:, :] = 0
    out[:, :, :, VALID:] = 0
    return out, res


def kernel(x, K):
    out, _ = _run(x, K, trace=False)
    return out
